# revision 17
# baseline (speedup 1.0000x reference)
"""Multi-head attention Trainium2 kernel (B=4, S=2048, D=1024, H=16, HD=64).

Sharding: core c handles batch b = c // 2, head-group g = c % 2 (8 heads).
Each core computes its heads' Q/K/V projections, masked softmax attention
(bounded by max valid_len), and a row-parallel partial of the output
projection.  The host sums the two partials per batch.

Layout strategy (per core):
  - scores are computed TRANSPOSED: S.T[kpos, q] = K_h @ Q_h.T so that the
    valid_len mask is a per-partition bias fused into the ACT exp, and the
    attention*V matmul needs no transposes anywhere.
  - head pairs are row-packed on the PE (contraction dh=64 at array rows
    0-63 / 64-127 concurrently).
  - softmax denominators come from a ones-column appended to V (the extra
    PSUM row costs no matmul time).
  - no max-subtraction in softmax: scores are O(1) here, exp is safe.
"""

import os
import threading

import numpy as np
import ml_dtypes

B, S, D, H = 4, 2048, 1024, 16
HD = 64
P = 128
DG = 512          # head dims per core (8 heads)
NPAIR = 4         # head pairs per core
MASK_VALUE = -1e6
QT_TILES = S // 512   # 4 query tiles of 512
DC = D // P           # 8 contraction chunks for projections

_BF16 = ml_dtypes.bfloat16

_build_cache = {}


def _build_nc(nkc: int):
    """Build the Bass program, parameterized by number of 128-wide key chunks."""
    import concourse.bass as bass
    import concourse.bacc as bacc
    import concourse.tile as tile
    from concourse import mybir

    f32 = mybir.dt.float32
    f32r = mybir.dt.float32r
    bf16 = mybir.dt.bfloat16
    EXP = mybir.ActivationFunctionType.Exp

    vlp = nkc * P
    # K/V projection free-dim tiles over key positions
    kt_widths = []
    rem = vlp
    while rem > 0:
        w = min(512, rem)
        kt_widths.append(w)
        rem -= w

    nc = bacc.Bacc("TRN2", target_bir_lowering=False)

    xq = nc.dram_tensor("xq", [D, S], bf16, kind="ExternalInput")
    xk = nc.dram_tensor("xk", [D, vlp], bf16, kind="ExternalInput")
    xv = nc.dram_tensor("xv", [D, vlp], bf16, kind="ExternalInput")
    wq = nc.dram_tensor("wq", [D, DG], bf16, kind="ExternalInput")  # pre-scaled 1/8
    wk = nc.dram_tensor("wk", [D, DG], bf16, kind="ExternalInput")
    wv = nc.dram_tensor("wv", [D, DG], bf16, kind="ExternalInput")
    wo = nc.dram_tensor("wo", [DG, D], bf16, kind="ExternalInput")
    maskd = nc.dram_tensor("maskd", [P, nkc], f32, kind="ExternalInput")
    y = nc.dram_tensor("y", [S, D], f32, kind="ExternalOutput")

    xq_r = xq.rearrange("(dc p) s -> p dc s", p=P)
    xk_r = xk.rearrange("(dc p) s -> p dc s", p=P)
    xv_r = xv.rearrange("(dc p) s -> p dc s", p=P)
    wq_r = wq.rearrange("(dc p) m -> p dc m", p=P)
    wk_r = wk.rearrange("(dc p) m -> p dc m", p=P)
    wv_r = wv.rearrange("(dc p) m -> p dc m", p=P)
    wo_r = wo.rearrange("(pr p) o -> p pr o", p=P)
    y_r = y.rearrange("(sc p) o -> p sc o", p=P)

    with tile.TileContext(nc) as tc:
        with (
            tc.tile_pool(name="persist", bufs=1) as persist,
            tc.tile_pool(name="xstream", bufs=4) as xstream,
            tc.tile_pool(name="work", bufs=3) as work,
            tc.tile_pool(name="ps_big", bufs=2, space="PSUM") as ps_big,
            tc.tile_pool(name="ps_proj", bufs=2, space="PSUM") as ps_proj,
            tc.tile_pool(name="ps_av", bufs=1, space="PSUM") as ps_av,
        ):
            # ---- resident tensors -------------------------------------
            wq_sb = persist.tile([P, DC, DG], bf16)
            wk_sb = persist.tile([P, DC, DG], bf16)
            wv_sb = persist.tile([P, DC, DG], bf16)
            wo_sb = persist.tile([P, NPAIR, D], bf16)
            mask_sb = persist.tile([P, nkc], f32)
            qt_sb = persist.tile([P, NPAIR, S], bf16)       # Q.T per pair
            kt_sb = persist.tile([P, NPAIR, vlp], bf16)     # K.T per pair
            v_sb = persist.tile([P, nkc, NPAIR, 130], bf16)  # [VA|1|VB|1] blocks
            ot_sb = persist.tile([P, NPAIR, S], bf16)       # normalized attn out
            ones_sb = persist.tile([P, 64], bf16)

            nc.sync.dma_start(wv_sb, wv_r)
            nc.sync.dma_start(wq_sb, wq_r)
            nc.sync.dma_start(wk_sb, wk_r)
            nc.sync.dma_start(mask_sb, maskd[:, :])
            nc.sync.dma_start(wo_sb, wo_r)
            nc.vector.memset(ones_sb, 1.0)
            # ones columns of the augmented V blocks
            nc.vector.memset(v_sb[:, :, :, 64:65], 1.0)
            nc.vector.memset(v_sb[:, :, :, 129:130], 1.0)

            # ---- V projection: V[s, dh] (s on partitions) -------------
            for vt in range(0, nkc, 4):
                nsc = min(4, nkc - vt)
                xv_t = xstream.tile([P, DC, 512], bf16, tag="xq")
                nc.gpsimd.dma_start(
                    xv_t[:, :, :nsc * P],
                    xv_r[:, :, vt * P:(vt + nsc) * P],
                )
                for s2 in range(nsc):
                    sc = vt + s2
                    v_ps = ps_proj.tile([P, DG], f32, tag="proj")
                    for dc in range(DC):
                        nc.tensor.matmul(
                            v_ps,
                            lhsT=xv_t[:, dc, s2 * P:(s2 + 1) * P],
                            rhs=wv_sb[:, dc, :],
                            start=(dc == 0), stop=(dc == DC - 1),
                        )
                    v_ps_h = v_ps.rearrange("p (pr h d) -> p pr h d", pr=NPAIR, h=2)
                    nc.vector.tensor_copy(
                        out=v_sb[:, sc, :, 0:64], in_=v_ps_h[:, :, 0, :]
                    )
                    nc.vector.tensor_copy(
                        out=v_sb[:, sc, :, 65:129], in_=v_ps_h[:, :, 1, :]
                    )

            def project_pair(pr):
                """Q.T and K.T projection for one head pair."""
                for st in range(QT_TILES):
                    xq_t = xstream.tile([P, DC, 512], bf16, tag="xq")
                    nc.gpsimd.dma_start(
                        xq_t, xq_r[:, :, st * 512:(st + 1) * 512]
                    )
                    q_ps = ps_proj.tile([P, 512], f32, tag="proj")
                    for dc in range(DC):
                        nc.tensor.matmul(
                            q_ps, lhsT=wq_sb[:, dc, pr * P:(pr + 1) * P],
                            rhs=xq_t[:, dc, :],
                            start=(dc == 0), stop=(dc == DC - 1),
                        )
                    nc.vector.tensor_copy(
                        out=qt_sb[:, pr, st * 512:(st + 1) * 512], in_=q_ps
                    )
                for kt, w in enumerate(kt_widths):
                    xk_t = xstream.tile([P, DC, 512], bf16, tag="xq")
                    nc.gpsimd.dma_start(
                        xk_t[:, :, :w], xk_r[:, :, kt * 512:kt * 512 + w]
                    )
                    k_ps = ps_proj.tile([P, 512], f32, tag="proj")
                    for dc in range(DC):
                        nc.tensor.matmul(
                            k_ps[:, :w], lhsT=wk_sb[:, dc, pr * P:(pr + 1) * P],
                            rhs=xk_t[:, dc, :w],
                            start=(dc == 0), stop=(dc == DC - 1),
                        )
                    nc.vector.tensor_copy(
                        out=kt_sb[:, pr, kt * 512:kt * 512 + w], in_=k_ps[:, :w]
                    )

            def make_norm(pr, qt, ut_sb):
                """Deferred normalization closure for one (pair, qtile)."""
                qsl = slice(qt * 512, (qt + 1) * 512)

                def norm():
                    # row 64 holds the softmax denominators: broadcast them
                    # across 64 partitions via PE ones-matmul, reciprocal,
                    # then scale the attention outputs.
                    dn_bf = work.tile([65, 1024], bf16, tag="dnbf", name="dn_bf")
                    nc.vector.tensor_copy(
                        out=dn_bf[64:65, :], in_=ut_sb[64:65, :]
                    )
                    bcA_ps = ps_proj.tile([64, 512], f32, tag="proj", name="bcA_ps")
                    nc.tensor.matmul(
                        bcA_ps, lhsT=ones_sb[64:65, 0:64],
                        rhs=dn_bf[64:65, 0:512],
                    )
                    bcB_ps = ps_proj.tile([64, 512], f32, tag="proj", name="bcB_ps")
                    nc.tensor.matmul(
                        bcB_ps, lhsT=ones_sb[64:65, 0:64],
                        rhs=dn_bf[64:65, 512:1024],
                    )
                    bc_sb = work.tile([64, 1024], f32, tag="bc", name="bc_sb")
                    nc.vector.reciprocal_approx_fast(
                        out=bc_sb[:, 0:512], in_=bcA_ps
                    )
                    nc.vector.reciprocal_approx_fast(
                        out=bc_sb[:, 512:1024], in_=bcB_ps
                    )
                    nc.vector.tensor_mul(
                        out=ot_sb[0:64, pr, qsl],
                        in0=ut_sb[0:64, 0:512], in1=bc_sb[:, 0:512],
                    )
                    otB = work.tile([64, 512], bf16, tag="otB", name="otB")
                    nc.vector.tensor_mul(
                        out=otB,
                        in0=ut_sb[0:64, 512:1024], in1=bc_sb[:, 512:1024],
                    )
                    nc.sync.dma_start(out=ot_sb[64:128, pr, qsl], in_=otB)

                return norm

            pending = [None]

            def attend(pr, qt, fillers):
                """Attention for one (pair, qtile); `fillers` are emitted at
                spaced points inside the kc loop so the PE fills ACT-bound
                gaps with projection / output-projection matmuls."""
                qsl = slice(qt * 512, (qt + 1) * 512)
                av_ps = ps_av.tile([65, 1024], f32, tag="av")
                fill_at = {}
                todo = []
                if fillers:
                    pts = [5 + (i * max(1, (nkc - 7)) // len(fillers))
                           for i in range(len(fillers))]
                    for p, (dma_fn, halves) in zip(pts, fillers):
                        if dma_fn is not None:
                            fill_at.setdefault(max(0, p - 3), []).append(dma_fn)
                            todo.append(dma_fn)
                        fill_at.setdefault(p, []).append(halves[0])
                        fill_at.setdefault(p + 1, []).append(halves[1])
                        todo += [halves[0], halves[1]]
                norm_at = min(3, nkc - 1)
                done = []
                for kc in range(nkc):
                    ksl = slice(kc * P, (kc + 1) * P)
                    sc_ps = ps_big.tile([P, 1024], f32, tag="big")
                    # scores.T for head A (rows 0-63) and B (rows 64-127)
                    nc.tensor.matmul(
                        sc_ps[:, 0:512],
                        lhsT=kt_sb[0:64, pr, ksl], rhs=qt_sb[0:64, pr, qsl],
                    )
                    nc.tensor.matmul(
                        sc_ps[:, 512:1024],
                        lhsT=kt_sb[64:128, pr, ksl], rhs=qt_sb[64:128, pr, qsl],
                    )
                    exps = work.tile([P, 1024], bf16, tag="exps", bufs=6)
                    nc.scalar.activation(
                        out=exps, in_=sc_ps, func=EXP,
                        bias=mask_sb[:, kc:kc + 1], scale=1.0,
                    )
                    nc.tensor.matmul(
                        av_ps[0:65, 0:512],
                        lhsT=v_sb[:, kc, pr, 0:65], rhs=exps[:, 0:512],
                        start=(kc == 0), stop=(kc == nkc - 1),
                    )
                    nc.tensor.matmul(
                        av_ps[0:65, 512:1024],
                        lhsT=v_sb[:, kc, pr, 65:130], rhs=exps[:, 512:1024],
                        start=(kc == 0), stop=(kc == nkc - 1),
                    )
                    if kc == norm_at and pending[0] is not None:
                        pending[0]()
                        pending[0] = None
                    for fl in fill_at.get(kc, ()):
                        fl()
                        done.append(fl)
                if pending[0] is not None:
                    pending[0]()
                    pending[0] = None
                for fl in todo:
                    if fl not in done:
                        fl()
                # drain the AV accumulator to SBUF right away so the PSUM
                # bank frees before the (lazy) normalization chain runs
                ut_sb = work.tile([65, 1024], f32, tag="ut", name="ut_sb")
                nc.vector.tensor_copy(out=ut_sb, in_=av_ps)
                pending[0] = make_norm(pr, qt, ut_sb)

            def q_dma(pr, st):
                xq_t = xstream.tile([P, DC, 512], bf16, tag="xq", name="xq_t")
                nc.gpsimd.dma_start(
                    xq_t, xq_r[:, :, st * 512:(st + 1) * 512]
                )
                return xq_t

            def proj_q_chunk(pr, st, xq_t, half, q_ps=None):
                if q_ps is None:
                    q_ps = ps_proj.tile([P, 512], f32, tag="proj", name="q_ps")
                for dc in range(half * 4, half * 4 + 4):
                    nc.tensor.matmul(
                        q_ps, lhsT=wq_sb[:, dc, pr * P:(pr + 1) * P],
                        rhs=xq_t[:, dc, :],
                        start=(dc == 0), stop=(dc == DC - 1),
                    )
                if half == 1:
                    nc.vector.tensor_copy(
                        out=qt_sb[:, pr, st * 512:(st + 1) * 512], in_=q_ps
                    )
                return q_ps

            def k_dma(pr, kt):
                w = kt_widths[kt]
                xk_t = xstream.tile([P, DC, 512], bf16, tag="xq", name="xk_t")
                nc.gpsimd.dma_start(
                    xk_t[:, :, :w], xk_r[:, :, kt * 512:kt * 512 + w]
                )
                return xk_t

            def proj_k_chunk(pr, kt, xk_t, half, k_ps=None):
                w = kt_widths[kt]
                if k_ps is None:
                    k_ps = ps_proj.tile([P, 512], f32, tag="proj", name="k_ps")
                for dc in range(half * 4, half * 4 + 4):
                    nc.tensor.matmul(
                        k_ps[:, :w], lhsT=wk_sb[:, dc, pr * P:(pr + 1) * P],
                        rhs=xk_t[:, dc, :w],
                        start=(dc == 0), stop=(dc == DC - 1),
                    )
                if half == 1:
                    nc.vector.tensor_copy(
                        out=kt_sb[:, pr, kt * 512:kt * 512 + w], in_=k_ps[:, :w]
                    )
                return k_ps

            def wo_tile(sch, half, ys=None):
                """Output projection partial for one 128-row chunk of s."""
                if ys is None:
                    y0 = ps_proj.tile([P, 512], f32, tag="proj", name="y0")
                    y1 = ps_proj.tile([P, 512], f32, tag="proj", name="y1")
                    ys = (y0, y1)
                y0, y1 = ys
                for pr in range(half * 2, half * 2 + 2):
                    nc.tensor.matmul(
                        y0, lhsT=ot_sb[:, pr, sch * P:(sch + 1) * P],
                        rhs=wo_sb[:, pr, 0:512],
                        start=(pr == 0), stop=(pr == NPAIR - 1),
                    )
                    nc.tensor.matmul(
                        y1, lhsT=ot_sb[:, pr, sch * P:(sch + 1) * P],
                        rhs=wo_sb[:, pr, 512:1024],
                        start=(pr == 0), stop=(pr == NPAIR - 1),
                    )
                if half == 1:
                    y_sb = work.tile([P, D], f32, tag="ysb", name="y_sb")
                    nc.vector.tensor_copy(out=y_sb[:, 0:512], in_=y0)
                    nc.vector.tensor_copy(out=y_sb[:, 512:1024], in_=y1)
                    nc.sync.dma_start(out=y_r[:, sch, :], in_=y_sb)
                return ys

            # ---- interleaved schedule ---------------------------------
            # Project pair 0 up front; the later pairs' projections and
            # the output projection are emitted as fillers inside the
            # ACT-bound attention loops.  Each filler is a two-stage
            # (prefetch-DMA, matmuls) pair so the PE never waits on HBM.
            for st in range(QT_TILES):
                t = q_dma(0, st)
                proj_q_chunk(0, st, t, 1, proj_q_chunk(0, st, t, 0))
            for kt in range(len(kt_widths)):
                t = k_dma(0, kt)
                proj_k_chunk(0, kt, t, 1, proj_k_chunk(0, kt, t, 0))

            def _mk2(dma_fn, mm_fn, *a):
                """Three-stage filler: prefetch DMA, then two 4-matmul halves."""
                state = {}

                def dma():
                    state["t"] = dma_fn(*a)

                def h0():
                    state["ps"] = mm_fn(*a, state["t"], 0)

                def h1():
                    mm_fn(*a, state["t"], 1, state["ps"])
                return dma, (h0, h1)

            def _mkwo(sch):
                state = {}

                def h0():
                    state["ps"] = wo_tile(sch, 0)

                def h1():
                    wo_tile(sch, 1, state["ps"])
                return None, (h0, h1)

            for pr in range(NPAIR):
                if pr + 1 < NPAIR:
                    chunks = (
                        [_mk2(q_dma, proj_q_chunk, pr + 1, st)
                         for st in range(QT_TILES)]
                        + [_mk2(k_dma, proj_k_chunk, pr + 1, kt)
                           for kt in range(len(kt_widths))]
                    )
                else:
                    chunks = [_mkwo(sch) for sch in range(12)]
                nchunk = len(chunks)
                for qt in range(QT_TILES):
                    if pr + 1 < NPAIR:
                        lo = nchunk * qt // QT_TILES
                        hi = nchunk * (qt + 1) // QT_TILES
                        sel = chunks[lo:hi]
                    else:
                        # Wo chunks for qtile q2 may only run after qtile
                        # q2-1's normalization, i.e. inside qtile q2.
                        sel = chunks[(qt - 1) * 4:qt * 4] if qt >= 1 else []
                    attend(pr, qt, sel)
            pending[0]()
            for sch in range(12, 16):
                wo_tile(sch, 1, wo_tile(sch, 0))

    nc.finalize()
    return nc


def _prep_core_inputs(inputs, nkc):
    """Host-side shard prep: per-core input dict for core c = (batch, group)."""
    vlp = nkc * P
    q = np.asarray(inputs["queries"], np.float32)
    k = np.asarray(inputs["keys"], np.float32)
    v = np.asarray(inputs["values"], np.float32)
    vl = np.asarray(inputs["valid_lens"]).astype(np.int64)
    Wq = np.asarray(inputs["Wq"], np.float32)
    Wk = np.asarray(inputs["Wk"], np.float32)
    Wv = np.asarray(inputs["Wv"], np.float32)
    Wo = np.asarray(inputs["Wo"], np.float32)

    in_maps = []
    for c in range(8):
        b, g = c // 2, c % 2
        rows = slice(g * DG, (g + 1) * DG)
        kpos = np.arange(P)[:, None] + P * np.arange(nkc)[None, :]
        mask = np.where(kpos < vl[b], 0.0, MASK_VALUE).astype(np.float32)
        in_maps.append({
            "xq": np.ascontiguousarray(q[b].T).astype(_BF16),
            "xk": np.ascontiguousarray(k[b, :vlp].T).astype(_BF16),
            "xv": np.ascontiguousarray(v[b, :vlp].T).astype(_BF16),
            "wq": np.ascontiguousarray(Wq[rows].T / 8.0).astype(_BF16),
            "wk": np.ascontiguousarray(Wk[rows].T).astype(_BF16),
            "wv": np.ascontiguousarray(Wv[rows].T).astype(_BF16),
            "wo": np.ascontiguousarray(Wo[:, rows].T).astype(_BF16),
            "maskd": mask,
        })
    return in_maps


def kernel(**inputs):
    from concourse.bass_utils import run_bass_kernel_spmd

    vl = np.asarray(inputs["valid_lens"]).astype(np.int64)
    nkc = int(min(S, max(1, int(vl.max()))) + P - 1) // P

    if nkc not in _build_cache:
        _build_cache[nkc] = _build_nc(nkc)
    nc = _build_cache[nkc]

    trace = bool(int(os.environ.get("MHA_TRACE", "0")))
    if trace:
        try:
            import antenv.axon_hooks  # noqa: F401
        except ImportError:
            trace = False

    in_maps = _prep_core_inputs(inputs, nkc)
    res = run_bass_kernel_spmd(
        nc, in_maps, core_ids=list(range(8)), trace=trace,
    )
    out = np.empty((B, S, D), np.float32)
    for b in range(B):
        out[b] = res.results[2 * b]["y"] + res.results[2 * b + 1]["y"]
    kernel.last_results = res
    return out


if __name__ == "__main__":
    rng = np.random.default_rng(0)
    ins = {
        "queries": rng.standard_normal((B, S, D), np.float32),
        "keys": rng.standard_normal((B, S, D), np.float32),
        "values": rng.standard_normal((B, S, D), np.float32),
        "valid_lens": np.array([288, 576, 1749, 255], np.int32),
        "Wq": rng.uniform(-1 / 32, 1 / 32, (D, D)).astype(np.float32),
        "Wk": rng.uniform(-1 / 32, 1 / 32, (D, D)).astype(np.float32),
        "Wv": rng.uniform(-1 / 32, 1 / 32, (D, D)).astype(np.float32),
        "Wo": rng.uniform(-1 / 32, 1 / 32, (D, D)).astype(np.float32),
    }
    out = kernel(**ins)
    print("kernel ran, out", out.shape, out.dtype, float(np.abs(out).mean()))
